# revision 5
# baseline (speedup 1.0000x reference)
"""Trainium2 Bass kernel for nn_MultiHeadGraphAttention.

Reference computation (B=4, N=2048, D=256, H=8, DK=32):
    Q = x @ w_q.T ; K = x @ w_k.T ; V = x @ w_v.T        (split into 8 heads of 32)
    scores = (Q K^T)/sqrt(32) + edge_weights, masked where mask==0
    out = softmax(scores) V  -> merge heads -> @ w_o.T + b_o

Sharding: 8 cores = batch(4) x sequence-halves(2). Each core owns batch b,
rows n0..n0+1023 and produces the full [1024, 256] output slab for them.

Per-core device algorithm (everything in "transposed" layout [feature/key, seq]):
    numerator = exp(QK^T/s) * (exp(edge) * mask)   -- the edge/mask factor EM is
    built once per (m,n) tile (2.1M elems) instead of per head (16.8M elems).
    Scores are computed transposed (scores_T[m, n]) so attention @ V streams
    directly on the PE; softmax denominators come from a ones-stationary matmul
    that lands pre-broadcast (32 rows/head) in PSUM; normalization is applied to
    the tiny [256, 1024] head-output instead of the huge attention matrix.
"""

import sys

for _p in ("/opt/trn_rl_repo", "/root/.axon_site/_ro/trn_rl_repo"):
    if _p not in sys.path:
        sys.path.insert(0, _p)

import numpy as np
import ml_dtypes

import concourse.bass as bass
import concourse.mybir as mybir
import concourse.tile as tile
from concourse.bass_utils import run_bass_kernel_spmd

B, N, D, H, DK = 4, 2048, 256, 8, 32
NL = N // 2          # rows per core
SCALE = float(np.sqrt(DK))
MB = N // 128        # 16 key blocks
NCH = NL // 512      # 2 query chunks of 512
F32 = mybir.dt.float32
BF16 = mybir.dt.bfloat16
I32 = mybir.dt.int32

_wait_ctr = [0]


def _split_multi_waits(nc, max_waits=1):
    """Walrus in this container rejects >1 sync wait per instruction; move
    extra waits onto NOPs inserted just before, on the same engine."""
    for fn in nc.m.functions:
        for bb in fn.blocks:
            insts = bb.instructions
            out = []
            changed = False
            for inst in insts:
                si = inst.sync_info
                if si is not None and len(si.on_wait) > max_waits:
                    waits = list(si.on_wait)
                    for w in waits[:-max_waits]:
                        _wait_ctr[0] += 1
                        out.append(
                            mybir.InstNoOp(
                                name=f"waitsplit-nop-{_wait_ctr[0]}",
                                engine=inst.engine,
                                sync_info=mybir.SyncInfo(on_wait=[w], on_update=[]),
                            )
                        )
                    inst.sync_info = mybir.SyncInfo(
                        on_wait=waits[-max_waits:], on_update=list(si.on_update)
                    )
                    changed = True
                out.append(inst)
            if changed:
                insts.clear()
                insts.extend(out)


def _build_program():
    nc = bass.Bass()

    xT = nc.dram_tensor("xT", [D, N], F32, kind="ExternalInput")
    xTq = nc.dram_tensor("xTq", [D, NL], F32, kind="ExternalInput")
    edgeT = nc.dram_tensor("edgeT", [N, NL], F32, kind="ExternalInput")
    maskT = nc.dram_tensor("maskT", [N, NL], I32, kind="ExternalInput")
    wqT = nc.dram_tensor("wqT", [D, D], F32, kind="ExternalInput")
    wkT = nc.dram_tensor("wkT", [D, D], F32, kind="ExternalInput")
    wvT = nc.dram_tensor("wvT", [D, D], F32, kind="ExternalInput")
    woT = nc.dram_tensor("woT", [D, D], F32, kind="ExternalInput")
    bo = nc.dram_tensor("bo", [1, D], F32, kind="ExternalInput")
    outd = nc.dram_tensor("out", [NL, D], F32, kind="ExternalOutput")

    with tile.TileContext(nc) as tc:
        with (
            tc.tile_pool(name="singles", bufs=1) as singles,
            tc.tile_pool(name="persist", bufs=1) as persist,
        ):
            # ---- static tiles -------------------------------------------------
            xT_sb = [singles.tile([128, N], F32, name=f"xt{p}") for p in range(2)]
            xTq_sb = [singles.tile([128, NL], F32, name=f"xtq{p}") for p in range(2)]
            wq_sb = [singles.tile([128, D], F32, name=f"wq{p}") for p in range(2)]
            wk_sb = [singles.tile([128, D], F32, name=f"wk{p}") for p in range(2)]
            wv_sb = [singles.tile([128, D], F32, name=f"wv{p}") for p in range(2)]
            wo_sb = [singles.tile([128, D], F32, name=f"wo{p}") for p in range(2)]
            bo_sb = singles.tile([128, D], F32, name="bo_sb")
            ones_sb = singles.tile([128, DK], BF16, name="ones_sb")

            for p in range(2):
                nc.sync.dma_start(out=xT_sb[p][:], in_=xT[p * 128:(p + 1) * 128, :])
                nc.sync.dma_start(out=xTq_sb[p][:], in_=xTq[p * 128:(p + 1) * 128, :])
                nc.sync.dma_start(out=wq_sb[p][:], in_=wqT[p * 128:(p + 1) * 128, :])
                nc.sync.dma_start(out=wk_sb[p][:], in_=wkT[p * 128:(p + 1) * 128, :])
                nc.sync.dma_start(out=wv_sb[p][:], in_=wvT[p * 128:(p + 1) * 128, :])
                nc.sync.dma_start(out=wo_sb[p][:], in_=woT[p * 128:(p + 1) * 128, :])
            nc.gpsimd.dma_start(out=bo_sb[:], in_=bo[0:1, :].partition_broadcast(128))
            nc.vector.memset(ones_sb[:], 1.0)

            # persistent intermediates
            QT_sb = [persist.tile([128, NL], F32, name=f"qt{p}") for p in range(2)]
            KT_sb = [persist.tile([128, N], F32, name=f"kt{p}") for p in range(2)]
            V_sb = persist.tile([128, MB, D], BF16, name="v_sb")
            emT_sb = persist.tile([128, MB, NL], BF16, name="emt_sb")
            houtT = [persist.tile([128, NL], F32, name=f"ho{g}") for g in range(2)]

            # ---- phase B: QKV projections ------------------------------------
            with tc.tile_pool(name="qkvps", bufs=2, space="PSUM") as qkvps:
                for p in range(2):
                    for f in range(NCH):
                        qps = qkvps.tile([128, 512], F32, name="qps", tag="qkv")
                        for dc in range(2):
                            nc.tensor.matmul(
                                qps[:],
                                wq_sb[dc][:, p * 128:(p + 1) * 128],
                                xTq_sb[dc][:, f * 512:(f + 1) * 512],
                                start=(dc == 0), stop=(dc == 1),
                            )
                        nc.vector.tensor_copy(
                            QT_sb[p][:, f * 512:(f + 1) * 512], qps[:]
                        )
                for p in range(2):
                    for f in range(4):
                        kps = qkvps.tile([128, 512], F32, name="kps", tag="qkv")
                        for dc in range(2):
                            nc.tensor.matmul(
                                kps[:],
                                wk_sb[dc][:, p * 128:(p + 1) * 128],
                                xT_sb[dc][:, f * 512:(f + 1) * 512],
                                start=(dc == 0), stop=(dc == 1),
                            )
                        nc.vector.tensor_copy(
                            KT_sb[p][:, f * 512:(f + 1) * 512], kps[:]
                        )
                for mb in range(MB):
                    vps = qkvps.tile([128, D], F32, name="vps", tag="qkv")
                    for dc in range(2):
                        nc.tensor.matmul(
                            vps[:],
                            xT_sb[dc][:, mb * 128:(mb + 1) * 128],
                            wv_sb[dc][:],
                            start=(dc == 0), stop=(dc == 1),
                        )
                    nc.vector.tensor_copy(V_sb[:, mb, :], vps[:])

            # ---- phase C: EM = exp(edge^T) * mask^T (bf16, [m, n] layout) ----
            with tc.tile_pool(name="emtrans", bufs=3) as emtrans:
                for mb in range(MB):
                    et = emtrans.tile([128, NL], F32, name="et", tag="et")
                    nc.sync.dma_start(
                        out=et[:], in_=edgeT[mb * 128:(mb + 1) * 128, :]
                    )
                    mt = emtrans.tile([128, NL], BF16, name="mt", tag="mt")
                    nc.gpsimd.dma_start(
                        out=mt[:], in_=maskT[mb * 128:(mb + 1) * 128, :]
                    )
                    ee = emtrans.tile([128, NL], BF16, name="ee", tag="ee")
                    nc.scalar.activation(
                        ee[:], et[:], mybir.ActivationFunctionType.Exp,
                        bias=0.0, scale=1.0,
                    )
                    nc.vector.tensor_mul(emT_sb[:, mb, :], ee[:], mt[:])

            # ---- phase D: attention main loop --------------------------------
            with (
                tc.tile_pool(name="spool", bufs=1, space="PSUM") as spool,
                tc.tile_pool(name="avpool", bufs=1, space="PSUM") as avpool,
                tc.tile_pool(name="dnpool", bufs=1, space="PSUM") as dnpool,
                tc.tile_pool(name="numpool", bufs=3) as numpool,
                tc.tile_pool(name="rcppool", bufs=2) as rcppool,
            ):
                for nch in range(NCH):
                    nsl = slice(nch * 512, (nch + 1) * 512)
                    avps = [
                        avpool.tile([128, 512], F32, name=f"av{g}", tag=f"av{g}")
                        for g in range(2)
                    ]
                    dnps = [
                        dnpool.tile([128, 512], F32, name=f"dn{g}", tag=f"dn{g}")
                        for g in range(2)
                    ]
                    for mb in range(MB):
                        for hg in range(2):
                            sps = spool.tile([128, 2048], F32, name="sps", tag="s")
                            for hh in range(4):
                                # scores_T[m,n] = sum_dk K_T[dk,m] * Q_T[dk,n]
                                nc.tensor.matmul(
                                    sps[:, hh * 512:(hh + 1) * 512],
                                    KT_sb[hg][hh * 32:(hh + 1) * 32,
                                              mb * 128:(mb + 1) * 128],
                                    QT_sb[hg][hh * 32:(hh + 1) * 32, nsl],
                                    start=True, stop=True,
                                    tile_position=(32 * hh, 0),
                                )
                            numer = numpool.tile(
                                [128, 2048], BF16, name="numer", tag="n"
                            )
                            nc.scalar.activation(
                                numer[:], sps[:],
                                mybir.ActivationFunctionType.Exp,
                                bias=0.0, scale=1.0,
                            )
                            for hh in range(4):
                                nc.vector.tensor_mul(
                                    numer[:, hh * 512:(hh + 1) * 512],
                                    numer[:, hh * 512:(hh + 1) * 512],
                                    emT_sb[:, mb, nsl],
                                )
                            for hh in range(4):
                                h = hg * 4 + hh
                                nc.tensor.matmul(
                                    avps[hg][32 * hh:32 * (hh + 1), :],
                                    V_sb[:, mb, h * 32:(h + 1) * 32],
                                    numer[:, hh * 512:(hh + 1) * 512],
                                    start=(mb == 0), stop=(mb == MB - 1),
                                    tile_position=(0, 32 * hh),
                                )
                                nc.tensor.matmul(
                                    dnps[hg][32 * hh:32 * (hh + 1), :],
                                    ones_sb[:, 0:32],
                                    numer[:, hh * 512:(hh + 1) * 512],
                                    start=(mb == 0), stop=(mb == MB - 1),
                                    tile_position=(0, 32 * hh),
                                )
                    for hg in range(2):
                        rcp = rcppool.tile([128, 512], F32, name="rcp", tag="rcp")
                        nc.vector.reciprocal(rcp[:], dnps[hg][:])
                        nc.vector.tensor_mul(houtT[hg][:, nsl], avps[hg][:], rcp[:])

            # ---- phase E: output projection + bias ---------------------------
            with (
                tc.tile_pool(name="outps", bufs=2, space="PSUM") as outps,
                tc.tile_pool(name="outpool", bufs=3) as outpool,
            ):
                for nb in range(NL // 128):
                    ops = outps.tile([128, D], F32, name="ops", tag="o")
                    for g in range(2):
                        nc.tensor.matmul(
                            ops[:],
                            houtT[g][:, nb * 128:(nb + 1) * 128],
                            wo_sb[g][:],
                            start=(g == 0), stop=(g == 1),
                        )
                    osb = outpool.tile([128, D], F32, name="osb", tag="osb")
                    nc.vector.tensor_add(osb[:], ops[:], bo_sb[:])
                    nc.sync.dma_start(
                        out=outd[nb * 128:(nb + 1) * 128, :], in_=osb[:]
                    )

    _split_multi_waits(nc)
    return nc


_NC_CACHE = None


def _get_program():
    global _NC_CACHE
    if _NC_CACHE is None:
        _NC_CACHE = _build_program()
    return _NC_CACHE


def _make_in_maps(x, edge_weights, mask, w_q, w_k, w_v, w_o, b_o):
    wqT = np.ascontiguousarray((w_q / SCALE).T).astype(np.float32)
    wkT = np.ascontiguousarray(w_k.T).astype(np.float32)
    wvT = np.ascontiguousarray(w_v.T).astype(np.float32)
    woT = np.ascontiguousarray(w_o.T).astype(np.float32)
    bo = np.ascontiguousarray(b_o.reshape(1, D)).astype(np.float32)
    in_maps = []
    for c in range(8):
        b, half = c // 2, c % 2
        n0 = half * NL
        xTb = np.ascontiguousarray(x[b].T).astype(np.float32)
        in_maps.append({
            "xT": xTb,
            "xTq": np.ascontiguousarray(xTb[:, n0:n0 + NL]),
            "edgeT": np.ascontiguousarray(edge_weights[b, n0:n0 + NL, :].T).astype(np.float32),
            "maskT": np.ascontiguousarray(mask[b, n0:n0 + NL, :].T).astype(np.int32),
            "wqT": wqT, "wkT": wkT, "wvT": wvT, "woT": woT, "bo": bo,
        })
    return in_maps


def run_sharded(inputs, trace=False, tmpdir=None):
    """Run the SPMD kernel; returns (full_output, BassKernelResults)."""
    arrs = {k: np.asarray(v) for k, v in inputs.items()}
    nc = _get_program()
    in_maps = _make_in_maps(**arrs)
    res = run_bass_kernel_spmd(
        nc, in_maps, list(range(8)), trace=trace, tmpdir=tmpdir
    )
    out = np.empty((B, N, D), np.float32)
    for c in range(8):
        b, half = c // 2, c % 2
        out[b, half * NL:(half + 1) * NL, :] = res.results[c]["out"]
    return out, res


def kernel(**inputs):
    out, _ = run_sharded(inputs, trace=False)
    return out


# revision 8
# speedup vs baseline: 1.6940x; 1.6940x over previous
"""Trainium2 Bass kernel for nn_MultiHeadGraphAttention.

Reference computation (B=4, N=2048, D=256, H=8, DK=32):
    Q = x @ w_q.T ; K = x @ w_k.T ; V = x @ w_v.T        (split into 8 heads of 32)
    scores = (Q K^T)/sqrt(32) + edge_weights, masked where mask==0
    out = softmax(scores) V  -> merge heads -> @ w_o.T + b_o

Sharding: 8 cores = batch(4) x sequence-halves(2). Each core owns batch b,
rows n0..n0+1023 and produces the full [1024, 256] output slab for them.

Per-core device algorithm (everything in "transposed" layout [feature/key, seq]):
    numerator = exp(QK^T/s) * (exp(edge) * mask)   -- the edge/mask factor EM is
    built once per (m,n) tile (2.1M elems) instead of per head (16.8M elems).
    Scores are computed transposed (scores_T[m, n]) so attention @ V streams
    directly on the PE; softmax denominators come from a ones-stationary matmul
    that lands pre-broadcast (32 rows/head) in PSUM; normalization is applied to
    the tiny [256, 1024] head-output instead of the huge attention matrix.
"""

import sys

for _p in ("/opt/trn_rl_repo", "/root/.axon_site/_ro/trn_rl_repo"):
    if _p not in sys.path:
        sys.path.insert(0, _p)

import numpy as np
import ml_dtypes

import concourse.bass as bass
import concourse.mybir as mybir
import concourse.tile as tile
from concourse.bass_utils import run_bass_kernel_spmd

B, N, D, H, DK = 4, 2048, 256, 8, 32
NL = N // 2          # rows per core
SCALE = float(np.sqrt(DK))
MB = N // 128        # 16 key blocks
NCH = NL // 512      # 2 query chunks of 512
F32 = mybir.dt.float32
BF16 = mybir.dt.bfloat16
I32 = mybir.dt.int32

_wait_ctr = [0]


def _split_multi_waits(nc, max_waits=1):
    """Walrus in this container rejects >1 sync wait per instruction; move
    extra waits onto NOPs inserted just before, on the same engine."""
    for fn in nc.m.functions:
        for bb in fn.blocks:
            insts = bb.instructions
            out = []
            changed = False
            for inst in insts:
                si = inst.sync_info
                if si is not None and len(si.on_wait) > max_waits:
                    waits = list(si.on_wait)
                    for w in waits[:-max_waits]:
                        _wait_ctr[0] += 1
                        out.append(
                            mybir.InstNoOp(
                                name=f"waitsplit-nop-{_wait_ctr[0]}",
                                engine=inst.engine,
                                sync_info=mybir.SyncInfo(on_wait=[w], on_update=[]),
                            )
                        )
                    inst.sync_info = mybir.SyncInfo(
                        on_wait=waits[-max_waits:], on_update=list(si.on_update)
                    )
                    changed = True
                out.append(inst)
            if changed:
                insts.clear()
                insts.extend(out)


def _build_program():
    nc = bass.Bass()

    xT = nc.dram_tensor("xT", [D, N], F32, kind="ExternalInput")
    xTq = nc.dram_tensor("xTq", [D, NL], F32, kind="ExternalInput")
    edgeT = nc.dram_tensor("edgeT", [N, NL], F32, kind="ExternalInput")
    maskT = nc.dram_tensor("maskT", [N, NL], I32, kind="ExternalInput")
    wqT = nc.dram_tensor("wqT", [D, D], F32, kind="ExternalInput")
    wkT = nc.dram_tensor("wkT", [D, D], F32, kind="ExternalInput")
    wvT = nc.dram_tensor("wvT", [D, D], F32, kind="ExternalInput")
    woT = nc.dram_tensor("woT", [D, D], F32, kind="ExternalInput")
    bo = nc.dram_tensor("bo", [1, D], F32, kind="ExternalInput")
    outd = nc.dram_tensor("out", [NL, D], F32, kind="ExternalOutput")

    with tile.TileContext(nc) as tc:
        with (
            tc.tile_pool(name="singles", bufs=1) as singles,
            tc.tile_pool(name="persist", bufs=1) as persist,
        ):
            # ---- static tiles -------------------------------------------------
            xT_sb = [singles.tile([128, N], F32, name=f"xt{p}") for p in range(2)]
            xTq_sb = [singles.tile([128, NL], F32, name=f"xtq{p}") for p in range(2)]
            wq_sb = [singles.tile([128, D], F32, name=f"wq{p}") for p in range(2)]
            wk_sb = [singles.tile([128, D], F32, name=f"wk{p}") for p in range(2)]
            wv_sb = [singles.tile([128, D], F32, name=f"wv{p}") for p in range(2)]
            wo_sb = [singles.tile([128, D], F32, name=f"wo{p}") for p in range(2)]
            bo_sb = singles.tile([128, D], F32, name="bo_sb")
            ones_sb = singles.tile([128, DK], BF16, name="ones_sb")

            for p in range(2):
                nc.sync.dma_start(out=xT_sb[p][:], in_=xT[p * 128:(p + 1) * 128, :])
                nc.sync.dma_start(out=xTq_sb[p][:], in_=xTq[p * 128:(p + 1) * 128, :])
                nc.sync.dma_start(out=wq_sb[p][:], in_=wqT[p * 128:(p + 1) * 128, :])
                nc.sync.dma_start(out=wk_sb[p][:], in_=wkT[p * 128:(p + 1) * 128, :])
                nc.sync.dma_start(out=wv_sb[p][:], in_=wvT[p * 128:(p + 1) * 128, :])
                nc.sync.dma_start(out=wo_sb[p][:], in_=woT[p * 128:(p + 1) * 128, :])
            nc.gpsimd.dma_start(out=bo_sb[:], in_=bo[0:1, :].partition_broadcast(128))
            nc.vector.memset(ones_sb[:], 1.0)

            # persistent intermediates (Q/K in bf16: halves PE stream cost)
            QT_sb = [persist.tile([128, NL], BF16, name=f"qt{p}") for p in range(2)]
            KT_sb = [persist.tile([128, N], BF16, name=f"kt{p}") for p in range(2)]
            V_sb = persist.tile([128, MB, D], BF16, name="v_sb")
            emT_sb = persist.tile([128, MB, NL], BF16, name="emt_sb")
            houtT = [persist.tile([128, NL], F32, name=f"ho{g}") for g in range(2)]

            # ---- phase B: QKV projections ------------------------------------
            with tc.tile_pool(name="qkvps", bufs=2, space="PSUM") as qkvps:
                for p in range(2):
                    for f in range(NCH):
                        qps = qkvps.tile([128, 512], F32, name="qps", tag="qkv")
                        for dc in range(2):
                            nc.tensor.matmul(
                                qps[:],
                                wq_sb[dc][:, p * 128:(p + 1) * 128],
                                xTq_sb[dc][:, f * 512:(f + 1) * 512],
                                start=(dc == 0), stop=(dc == 1),
                            )
                        nc.vector.tensor_copy(
                            QT_sb[p][:, f * 512:(f + 1) * 512], qps[:]
                        )
                for p in range(2):
                    for f in range(4):
                        kps = qkvps.tile([128, 512], F32, name="kps", tag="qkv")
                        for dc in range(2):
                            nc.tensor.matmul(
                                kps[:],
                                wk_sb[dc][:, p * 128:(p + 1) * 128],
                                xT_sb[dc][:, f * 512:(f + 1) * 512],
                                start=(dc == 0), stop=(dc == 1),
                            )
                        nc.vector.tensor_copy(
                            KT_sb[p][:, f * 512:(f + 1) * 512], kps[:]
                        )
                for mb in range(MB):
                    vps = qkvps.tile([128, D], F32, name="vps", tag="qkv")
                    for dc in range(2):
                        nc.tensor.matmul(
                            vps[:],
                            xT_sb[dc][:, mb * 128:(mb + 1) * 128],
                            wv_sb[dc][:],
                            start=(dc == 0), stop=(dc == 1),
                        )
                    nc.vector.tensor_copy(V_sb[:, mb, :], vps[:])

            # ---- phase C: EM = exp(edge^T) * mask^T (bf16, [m, n] layout) ----
            with tc.tile_pool(name="emtrans", bufs=3) as emtrans:
                for mb in range(MB):
                    et = emtrans.tile([128, NL], F32, name="et", tag="et")
                    nc.sync.dma_start(
                        out=et[:], in_=edgeT[mb * 128:(mb + 1) * 128, :]
                    )
                    mt = emtrans.tile([128, NL], BF16, name="mt", tag="mt")
                    nc.gpsimd.dma_start(
                        out=mt[:], in_=maskT[mb * 128:(mb + 1) * 128, :]
                    )
                    ee = emtrans.tile([128, NL], BF16, name="ee", tag="ee")
                    nc.scalar.activation(
                        ee[:], et[:], mybir.ActivationFunctionType.Exp,
                        bias=0.0, scale=1.0,
                    )
                    nc.vector.tensor_mul(emT_sb[:, mb, :], ee[:], mt[:])

            # ---- phase D: attention main loop --------------------------------
            with (
                tc.tile_pool(name="spool", bufs=2, space="PSUM") as spool,
                tc.tile_pool(name="avpool", bufs=1, space="PSUM") as avpool,
                tc.tile_pool(name="dnpool", bufs=1, space="PSUM") as dnpool,
                tc.tile_pool(name="numpool", bufs=3) as numpool,
                tc.tile_pool(name="rcppool", bufs=2) as rcppool,
            ):
                for nch in range(NCH):
                    nsl = slice(nch * 512, (nch + 1) * 512)
                    avps = [
                        avpool.tile([128, 512], F32, name=f"av{g}", tag=f"av{g}")
                        for g in range(2)
                    ]
                    dnps = [
                        dnpool.tile([128, 512], F32, name=f"dn{g}", tag=f"dn{g}")
                        for g in range(2)
                    ]
                    for mb in range(MB):
                        for grp in range(4):  # 2 heads per group
                            sps = spool.tile([128, 1024], F32, name="sps", tag="s")
                            for hh2 in range(2):
                                h = grp * 2 + hh2
                                # scores_T[m,n] = sum_dk K_T[dk,m] * Q_T[dk,n]
                                nc.tensor.matmul(
                                    sps[:, hh2 * 512:(hh2 + 1) * 512],
                                    KT_sb[h // 4][(h % 4) * 32:(h % 4 + 1) * 32,
                                                  mb * 128:(mb + 1) * 128],
                                    QT_sb[h // 4][(h % 4) * 32:(h % 4 + 1) * 32, nsl],
                                    start=True, stop=True,
                                    tile_position=(32 * (h % 4), 0),
                                )
                            numer = numpool.tile(
                                [128, 1024], BF16, name="numer", tag="n"
                            )
                            nc.scalar.activation(
                                numer[:], sps[:],
                                mybir.ActivationFunctionType.Exp,
                                bias=0.0, scale=1.0,
                            )
                            for hh2 in range(2):
                                nc.vector.tensor_mul(
                                    numer[:, hh2 * 512:(hh2 + 1) * 512],
                                    numer[:, hh2 * 512:(hh2 + 1) * 512],
                                    emT_sb[:, mb, nsl],
                                )
                            for hh2 in range(2):
                                h = grp * 2 + hh2
                                hg = h // 4
                                nc.tensor.matmul(
                                    avps[hg][32 * (h % 4):32 * (h % 4 + 1), :],
                                    V_sb[:, mb, h * 32:(h + 1) * 32],
                                    numer[:, hh2 * 512:(hh2 + 1) * 512],
                                    start=(mb == 0), stop=(mb == MB - 1),
                                    tile_position=(0, 32 * (h % 4)),
                                )
                                nc.tensor.matmul(
                                    dnps[hg][32 * (h % 4):32 * (h % 4 + 1), :],
                                    ones_sb[:, 0:32],
                                    numer[:, hh2 * 512:(hh2 + 1) * 512],
                                    start=(mb == 0), stop=(mb == MB - 1),
                                    tile_position=(0, 32 * (h % 4)),
                                )
                    for hg in range(2):
                        rcp = rcppool.tile([128, 512], F32, name="rcp", tag="rcp")
                        nc.vector.reciprocal(rcp[:], dnps[hg][:])
                        nc.vector.tensor_mul(houtT[hg][:, nsl], avps[hg][:], rcp[:])

            # ---- phase E: output projection + bias ---------------------------
            with (
                tc.tile_pool(name="outps", bufs=2, space="PSUM") as outps,
                tc.tile_pool(name="outpool", bufs=3) as outpool,
            ):
                for nb in range(NL // 128):
                    ops = outps.tile([128, D], F32, name="ops", tag="o")
                    for g in range(2):
                        nc.tensor.matmul(
                            ops[:],
                            houtT[g][:, nb * 128:(nb + 1) * 128],
                            wo_sb[g][:],
                            start=(g == 0), stop=(g == 1),
                        )
                    osb = outpool.tile([128, D], F32, name="osb", tag="osb")
                    nc.vector.tensor_add(osb[:], ops[:], bo_sb[:])
                    nc.sync.dma_start(
                        out=outd[nb * 128:(nb + 1) * 128, :], in_=osb[:]
                    )

    _split_multi_waits(nc)
    return nc


_NC_CACHE = None


def _get_program():
    global _NC_CACHE
    if _NC_CACHE is None:
        _NC_CACHE = _build_program()
    return _NC_CACHE


def _make_in_maps(x, edge_weights, mask, w_q, w_k, w_v, w_o, b_o):
    wqT = np.ascontiguousarray((w_q / SCALE).T).astype(np.float32)
    wkT = np.ascontiguousarray(w_k.T).astype(np.float32)
    wvT = np.ascontiguousarray(w_v.T).astype(np.float32)
    woT = np.ascontiguousarray(w_o.T).astype(np.float32)
    bo = np.ascontiguousarray(b_o.reshape(1, D)).astype(np.float32)
    in_maps = []
    for c in range(8):
        b, half = c // 2, c % 2
        n0 = half * NL
        xTb = np.ascontiguousarray(x[b].T).astype(np.float32)
        in_maps.append({
            "xT": xTb,
            "xTq": np.ascontiguousarray(xTb[:, n0:n0 + NL]),
            "edgeT": np.ascontiguousarray(edge_weights[b, n0:n0 + NL, :].T).astype(np.float32),
            "maskT": np.ascontiguousarray(mask[b, n0:n0 + NL, :].T).astype(np.int32),
            "wqT": wqT, "wkT": wkT, "wvT": wvT, "woT": woT, "bo": bo,
        })
    return in_maps


def run_sharded(inputs, trace=False, tmpdir=None):
    """Run the SPMD kernel; returns (full_output, BassKernelResults)."""
    arrs = {k: np.asarray(v) for k, v in inputs.items()}
    nc = _get_program()
    in_maps = _make_in_maps(**arrs)
    res = run_bass_kernel_spmd(
        nc, in_maps, list(range(8)), trace=trace, tmpdir=tmpdir
    )
    out = np.empty((B, N, D), np.float32)
    for c in range(8):
        b, half = c // 2, c % 2
        out[b, half * NL:(half + 1) * NL, :] = res.results[c]["out"]
    return out, res


def kernel(**inputs):
    out, _ = run_sharded(inputs, trace=False)
    return out


# revision 17
# speedup vs baseline: 1.8446x; 1.0889x over previous
"""Trainium2 Bass kernel for nn_MultiHeadGraphAttention.

Reference computation (B=4, N=2048, D=256, H=8, DK=32):
    Q = x @ w_q.T ; K = x @ w_k.T ; V = x @ w_v.T        (split into 8 heads of 32)
    scores = (Q K^T)/sqrt(32) + edge_weights, masked where mask==0
    out = softmax(scores) V  -> merge heads -> @ w_o.T + b_o

Sharding: 8 cores = batch(4) x sequence-halves(2). Each core owns batch b,
rows n0..n0+1023 and produces the full [1024, 256] output slab for them.

Per-core device algorithm (everything in "transposed" layout [feature/key, seq]):
    numerator = exp(QK^T/s) * (exp(edge) * mask)   -- the edge/mask factor EM is
    built once per (m,n) tile (2.1M elems) instead of per head (16.8M elems).
    Scores are computed transposed (scores_T[m, n]) so attention @ V streams
    directly on the PE; softmax denominators come from a ones-stationary matmul
    that lands pre-broadcast (32 rows/head) in PSUM; normalization is applied to
    the tiny [256, 1024] head-output instead of the huge attention matrix.
"""

import sys

for _p in ("/opt/trn_rl_repo", "/root/.axon_site/_ro/trn_rl_repo"):
    if _p not in sys.path:
        sys.path.insert(0, _p)

import numpy as np
import ml_dtypes

import concourse.bass as bass
import concourse.mybir as mybir
import concourse.tile as tile
from concourse.bass_utils import run_bass_kernel_spmd

B, N, D, H, DK = 4, 2048, 256, 8, 32
NL = N // 2          # rows per core
SCALE = float(np.sqrt(DK))
MB = N // 128        # 16 key blocks
NCH = NL // 512      # 2 query chunks of 512
F32 = mybir.dt.float32
BF16 = mybir.dt.bfloat16
I32 = mybir.dt.int32

_wait_ctr = [0]


def _split_multi_waits(nc, max_waits=1):
    """Walrus in this container rejects >1 sync wait per instruction; move
    extra waits onto NOPs inserted just before, on the same engine."""
    for fn in nc.m.functions:
        for bb in fn.blocks:
            insts = bb.instructions
            out = []
            changed = False
            for inst in insts:
                si = inst.sync_info
                if si is not None and len(si.on_wait) > max_waits:
                    waits = list(si.on_wait)
                    for w in waits[:-max_waits]:
                        _wait_ctr[0] += 1
                        out.append(
                            mybir.InstNoOp(
                                name=f"waitsplit-nop-{_wait_ctr[0]}",
                                engine=inst.engine,
                                sync_info=mybir.SyncInfo(on_wait=[w], on_update=[]),
                            )
                        )
                    inst.sync_info = mybir.SyncInfo(
                        on_wait=waits[-max_waits:], on_update=list(si.on_update)
                    )
                    changed = True
                out.append(inst)
            if changed:
                insts.clear()
                insts.extend(out)


def _build_program():
    nc = bass.Bass()

    xT = nc.dram_tensor("xT", [D, N], F32, kind="ExternalInput")
    xTq = nc.dram_tensor("xTq", [D, NL], F32, kind="ExternalInput")
    edgeT = nc.dram_tensor("edgeT", [N, NL], F32, kind="ExternalInput")
    maskT = nc.dram_tensor("maskT", [N, NL], I32, kind="ExternalInput")
    wqT = nc.dram_tensor("wqT", [D, D], F32, kind="ExternalInput")
    wkT = nc.dram_tensor("wkT", [D, D], F32, kind="ExternalInput")
    wvT = nc.dram_tensor("wvT", [D, D], F32, kind="ExternalInput")
    woT = nc.dram_tensor("woT", [D, D], F32, kind="ExternalInput")
    bo = nc.dram_tensor("bo", [1, D], F32, kind="ExternalInput")
    outd = nc.dram_tensor("out", [NL, D], F32, kind="ExternalOutput")

    with tile.TileContext(nc) as tc:
        with (
            tc.tile_pool(name="singles", bufs=1) as singles,
            tc.tile_pool(name="persist", bufs=1) as persist,
        ):
            # ---- static tiles -------------------------------------------------
            xT_sb = [singles.tile([128, N], BF16, name=f"xt{p}") for p in range(2)]
            xTq_sb = [singles.tile([128, NL], BF16, name=f"xtq{p}") for p in range(2)]
            wq_sb = [singles.tile([128, D], BF16, name=f"wq{p}") for p in range(2)]
            wk_sb = [singles.tile([128, D], BF16, name=f"wk{p}") for p in range(2)]
            wv_sb = [singles.tile([128, D], BF16, name=f"wv{p}") for p in range(2)]
            wo_sb = [singles.tile([128, D], F32, name=f"wo{p}") for p in range(2)]
            bo_sb = singles.tile([128, D], F32, name="bo_sb")
            ones128 = singles.tile([128, 32], F32, name="ones128")
            nc.vector.memset(ones128[:], 1.0)

            for p in range(2):
                nc.gpsimd.dma_start(out=xT_sb[p][:], in_=xT[p * 128:(p + 1) * 128, :])
                nc.gpsimd.dma_start(out=xTq_sb[p][:], in_=xTq[p * 128:(p + 1) * 128, :])
                nc.gpsimd.dma_start(out=wq_sb[p][:], in_=wqT[p * 128:(p + 1) * 128, :])
                nc.gpsimd.dma_start(out=wk_sb[p][:], in_=wkT[p * 128:(p + 1) * 128, :])
                nc.gpsimd.dma_start(out=wv_sb[p][:], in_=wvT[p * 128:(p + 1) * 128, :])
                nc.sync.dma_start(out=wo_sb[p][:], in_=woT[p * 128:(p + 1) * 128, :])
            nc.gpsimd.dma_start(out=bo_sb[:], in_=bo[0:1, :].partition_broadcast(128))

            # persistent intermediates (Q/K in bf16: halves PE stream cost)
            QT_sb = [persist.tile([128, NL], BF16, name=f"qt{p}") for p in range(2)]
            KT_sb = [persist.tile([128, N], BF16, name=f"kt{p}") for p in range(2)]
            # V augmented with a ones column: AV matmul (M=33) then yields both
            # attention@V (rows 0-31) and the softmax denominator (row 32).
            V_aug = persist.tile([128, MB, H, 64], BF16, name="v_aug")
            nc.vector.memset(V_aug[:], 0.0)
            nc.vector.memset(V_aug[:, :, :, 32:33], 1.0)
            emT_sb = persist.tile([128, MB, NL], BF16, name="emt_sb")
            houtT = [persist.tile([128, NL], F32, name=f"ho{g}") for g in range(2)]

            # ---- phase B: QKV projections ------------------------------------
            with tc.tile_pool(name="qkvps", bufs=2, space="PSUM") as qkvps:
                for p in range(2):
                    for f in range(NCH):
                        qps = qkvps.tile([128, 512], F32, name="qps", tag="qkv")
                        for dc in range(2):
                            nc.tensor.matmul(
                                qps[:],
                                wq_sb[dc][:, p * 128:(p + 1) * 128],
                                xTq_sb[dc][:, f * 512:(f + 1) * 512],
                                start=(dc == 0), stop=(dc == 1),
                            )
                        nc.vector.tensor_copy(
                            QT_sb[p][:, f * 512:(f + 1) * 512], qps[:]
                        )
                for p in range(2):
                    for f in range(4):
                        kps = qkvps.tile([128, 512], F32, name="kps", tag="qkv")
                        for dc in range(2):
                            nc.tensor.matmul(
                                kps[:],
                                wk_sb[dc][:, p * 128:(p + 1) * 128],
                                xT_sb[dc][:, f * 512:(f + 1) * 512],
                                start=(dc == 0), stop=(dc == 1),
                            )
                        nc.vector.tensor_copy(
                            KT_sb[p][:, f * 512:(f + 1) * 512], kps[:]
                        )
                for mb in range(MB):
                    vps = qkvps.tile([128, D], F32, name="vps", tag="qkv")
                    for dc in range(2):
                        nc.tensor.matmul(
                            vps[:],
                            xT_sb[dc][:, mb * 128:(mb + 1) * 128],
                            wv_sb[dc][:],
                            start=(dc == 0), stop=(dc == 1),
                        )
                    nc.vector.tensor_copy(
                        V_aug[:, mb, :, 0:32],
                        vps[:].rearrange("p (h d) -> p h d", h=H),
                    )

            # ---- phase C: EM = exp(edge^T) * mask^T (bf16, [m, n] layout) ----
            with tc.tile_pool(name="emtrans", bufs=3) as emtrans:
                for mb in range(MB):
                    et = emtrans.tile([128, NL], F32, name="et", tag="et")
                    nc.sync.dma_start(
                        out=et[:], in_=edgeT[mb * 128:(mb + 1) * 128, :]
                    )
                    mt = emtrans.tile([128, NL], BF16, name="mt", tag="mt")
                    nc.gpsimd.dma_start(
                        out=mt[:], in_=maskT[mb * 128:(mb + 1) * 128, :]
                    )
                    ee = emtrans.tile([128, NL], BF16, name="ee", tag="ee")
                    nc.scalar.activation(
                        ee[:], et[:], mybir.ActivationFunctionType.Exp,
                        bias=0.0, scale=1.0,
                    )
                    nc.vector.tensor_mul(emT_sb[:, mb, :], ee[:], mt[:])

            # ---- phase D: attention main loop --------------------------------
            with (
                tc.tile_pool(name="spool", bufs=2, space="PSUM") as spool,
                tc.tile_pool(name="avpool", bufs=1, space="PSUM") as avpool,
                tc.tile_pool(name="numpool", bufs=3) as numpool,
                tc.tile_pool(name="rcppool", bufs=2) as rcppool,
            ):
                for nch in range(NCH):
                    nsl = slice(nch * 512, (nch + 1) * 512)
                    # bank b holds heads (2b, 2b+1): rows 0-32 and 64-96
                    avps = [
                        avpool.tile([128, 512], F32, name=f"av{b}", tag=f"av{b}")
                        for b in range(4)
                    ]
                    for mb in range(MB):
                        for grp in range(4):  # 2 heads per group
                            sps = spool.tile([128, 1024], F32, name="sps", tag="s")
                            for hh2 in range(2):
                                h = grp * 2 + hh2
                                # scores_T[m,n] = sum_dk K_T[dk,m] * Q_T[dk,n]
                                nc.tensor.matmul(
                                    sps[:, hh2 * 512:(hh2 + 1) * 512],
                                    KT_sb[h // 4][(h % 4) * 32:(h % 4 + 1) * 32,
                                                  mb * 128:(mb + 1) * 128],
                                    QT_sb[h // 4][(h % 4) * 32:(h % 4 + 1) * 32, nsl],
                                    start=True, stop=True,
                                    tile_position=(32 * (h % 4), 0),
                                )
                            numer = numpool.tile(
                                [128, 1024], BF16, name="numer", tag="n"
                            )
                            nc.scalar.activation(
                                numer[:], sps[:],
                                mybir.ActivationFunctionType.Exp,
                                bias=0.0, scale=1.0,
                            )
                            for hh2 in range(2):
                                nc.vector.tensor_mul(
                                    numer[:, hh2 * 512:(hh2 + 1) * 512],
                                    numer[:, hh2 * 512:(hh2 + 1) * 512],
                                    emT_sb[:, mb, nsl],
                                )
                            for hh2 in range(2):
                                h = grp * 2 + hh2
                                b, sub = h // 2, h % 2
                                nc.tensor.matmul(
                                    avps[b][64 * sub:64 * sub + 33, :],
                                    V_aug[:, mb, h, 0:33],
                                    numer[:, hh2 * 512:(hh2 + 1) * 512],
                                    start=(mb == 0), stop=(mb == MB - 1),
                                    tile_position=(0, 64 * sub),
                                )
                    # normalize: recip of denominator rows (placed at 32-aligned
                    # partitions), PE contract-1 ones-matmul broadcasts each row
                    # to its 32-row head block, then multiply.
                    for hg in range(2):
                        rcps = rcppool.tile(
                            [128, 512], F32, name=f"rcps{hg}", tag=f"rcp{hg}"
                        )
                        for j in range(4):
                            h = hg * 4 + j
                            b, sub = h // 2, h % 2
                            nc.vector.reciprocal(
                                rcps[32 * j:32 * j + 1, :],
                                avps[b][64 * sub + 32:64 * sub + 33, :],
                            )
                        rcpb_ps = spool.tile(
                            [128, 512], F32, name="rcpb_ps", tag="s"
                        )
                        for j in range(4):
                            nc.tensor.matmul(
                                rcpb_ps[32 * j:32 * j + 32, :],
                                ones128[32 * j:32 * j + 1, 0:32],
                                rcps[32 * j:32 * j + 1, :],
                                start=True, stop=True,
                                tile_position=(32 * j, 32 * j),
                            )
                        rcpb_g = rcppool.tile(
                            [128, 512], F32, name=f"rcpb{hg}", tag=f"rcpb{hg}"
                        )
                        nc.vector.tensor_copy(rcpb_g[:], rcpb_ps[:])
                        for j in range(4):
                            h = hg * 4 + j
                            b, sub = h // 2, h % 2
                            nc.vector.tensor_mul(
                                houtT[hg][32 * j:32 * j + 32, nsl],
                                avps[b][64 * sub:64 * sub + 32, :],
                                rcpb_g[32 * j:32 * j + 32, :],
                            )

            # ---- phase E: output projection + bias ---------------------------
            with (
                tc.tile_pool(name="outps", bufs=2, space="PSUM") as outps,
                tc.tile_pool(name="outpool", bufs=3) as outpool,
            ):
                for nb in range(NL // 128):
                    ops = outps.tile([128, D], F32, name="ops", tag="o")
                    for g in range(2):
                        nc.tensor.matmul(
                            ops[:],
                            houtT[g][:, nb * 128:(nb + 1) * 128],
                            wo_sb[g][:],
                            start=(g == 0), stop=(g == 1),
                        )
                    osb = outpool.tile([128, D], F32, name="osb", tag="osb")
                    nc.vector.tensor_add(osb[:], ops[:], bo_sb[:])
                    nc.sync.dma_start(
                        out=outd[nb * 128:(nb + 1) * 128, :], in_=osb[:]
                    )

    _split_multi_waits(nc)
    return nc


_NC_CACHE = None


def _get_program():
    global _NC_CACHE
    if _NC_CACHE is None:
        _NC_CACHE = _build_program()
    return _NC_CACHE


def _make_in_maps(x, edge_weights, mask, w_q, w_k, w_v, w_o, b_o):
    wqT = np.ascontiguousarray((w_q / SCALE).T).astype(np.float32)
    wkT = np.ascontiguousarray(w_k.T).astype(np.float32)
    wvT = np.ascontiguousarray(w_v.T).astype(np.float32)
    woT = np.ascontiguousarray(w_o.T).astype(np.float32)
    bo = np.ascontiguousarray(b_o.reshape(1, D)).astype(np.float32)
    in_maps = []
    for c in range(8):
        b, half = c // 2, c % 2
        n0 = half * NL
        xTb = np.ascontiguousarray(x[b].T).astype(np.float32)
        in_maps.append({
            "xT": xTb,
            "xTq": np.ascontiguousarray(xTb[:, n0:n0 + NL]),
            "edgeT": np.ascontiguousarray(edge_weights[b, n0:n0 + NL, :].T).astype(np.float32),
            "maskT": np.ascontiguousarray(mask[b, n0:n0 + NL, :].T).astype(np.int32),
            "wqT": wqT, "wkT": wkT, "wvT": wvT, "woT": woT, "bo": bo,
        })
    return in_maps


def run_sharded(inputs, trace=False, tmpdir=None):
    """Run the SPMD kernel; returns (full_output, BassKernelResults)."""
    arrs = {k: np.asarray(v) for k, v in inputs.items()}
    nc = _get_program()
    in_maps = _make_in_maps(**arrs)
    res = run_bass_kernel_spmd(
        nc, in_maps, list(range(8)), trace=trace, tmpdir=tmpdir
    )
    out = np.empty((B, N, D), np.float32)
    for c in range(8):
        b, half = c // 2, c % 2
        out[b, half * NL:(half + 1) * NL, :] = res.results[c]["out"]
    return out, res


def kernel(**inputs):
    out, _ = run_sharded(inputs, trace=False)
    return out
